# revision 5
# baseline (speedup 1.0000x reference)
"""Multi-head attention (B=16, N=1024, D=768, H=12) on 8 TRN2 NeuronCores.

Strategy: data-parallel over batch (2 batches per core, no collectives).
Per-core kernel, all matmuls on TensorE:
  - QKV projection from pre-transposed x (feature-major xT in SBUF),
    fp32r (full-rate fp32-storage matmul mode).
  - Scores computed directly TRANSPOSED (S^T[k, q]) so the exp output
    P^T lands in exactly the layout the PV matmul needs as rhs; the two
    heads of a pair run concurrently on disjoint PE row groups (K=64).
  - exp on ScalarE with the 1/sqrt(hd) scale folded in (no max-subtract:
    scores are O(5) for this input distribution, far from fp32 overflow).
  - Softmax denominators via ones-matmul (M=1 outputs at partition 0/32
    of a shared PSUM bank), broadcast back over partitions with a tiny
    K=33 sel-matmul; the 1/denominator normalization is fused into the
    PV PSUM->SBUF copyback on VectorE.
  - PV col-tiled (two heads per PSUM bank, M=64 each) in bf16 producing
    O^T feature-major, which feeds the output projection (bf16) without
    any transposes.
"""

import sys

sys.path.insert(0, "/opt/trn_rl_repo")

import numpy as np
import ml_dtypes

import concourse.mybir as mybir
import concourse.tile as tile
from concourse import bacc
from concourse.bass_utils import run_bass_kernel_spmd

F32 = mybir.dt.float32
F32R = mybir.dt.float32r
BF16 = mybir.dt.bfloat16

B, N, D = 16, 1024, 768
H = 12
HD = D // H          # 64
SCALE = float(HD) ** -0.5   # 0.125
NCORES = 8
BL = B // NCORES     # batches per core
ROWS = BL * N        # 2048 rows per core
DT = D // 128        # 6 d-tiles
NP = H // 2          # 6 head pairs
EXP = mybir.ActivationFunctionType.Exp
MUL = mybir.AluOpType.mult
ADD = mybir.AluOpType.add


def build_nc(repeat=1):
    nc = bacc.Bacc("TRN2", target_bir_lowering=False, debug=False)

    xT_ext = nc.declare_dram_parameter("xT", [D, ROWS], F32, isOutput=False)
    wqkvT_ext = nc.declare_dram_parameter("wqkvT", [D, 3 * D], F32, isOutput=False)
    wprojT_ext = nc.declare_dram_parameter("wprojT", [D, D], BF16, isOutput=False)
    bias_ext = nc.declare_dram_parameter("biasb", [128, D], F32, isOutput=False)
    out_ext = nc.declare_dram_parameter("out", [ROWS, D], F32, isOutput=True)

    with tile.TileContext(nc) as tc:
        with (
            tc.tile_pool(name="const", bufs=1) as constp,
            tc.tile_pool(name="work", bufs=1) as work,
            tc.tile_pool(name="mmps", bufs=2, space="PSUM") as mmps,
            tc.tile_pool(name="stps", bufs=2, space="PSUM") as stps,
            tc.tile_pool(name="pvps", bufs=1, space="PSUM") as pvps,
            tc.tile_pool(name="denps", bufs=1, space="PSUM") as denps,
        ):
            # ---- constants ----
            wqkvT_sb = constp.tile([128, DT, 3 * D], F32R)
            nc.sync.dma_start(
                wqkvT_sb[:],
                wqkvT_ext.rearrange("(o p) e -> p o e", p=128).bitcast(F32R),
            )
            wprojT_sb = constp.tile([128, DT, D], BF16)
            nc.sync.dma_start(
                wprojT_sb[:], wprojT_ext.rearrange("(o p) e -> p o e", p=128)
            )
            bias_sb = constp.tile([128, D], F32)
            nc.sync.dma_start(bias_sb[:], bias_ext[:])
            sel_f = constp.tile([33, 128], F32)
            nc.vector.memset(sel_f[:], 0.0)
            nc.vector.memset(sel_f[0:1, 0:64], 1.0)
            nc.vector.memset(sel_f[32:33, 64:128], 1.0)
            sel_sb = constp.tile([33, 128], F32R)
            nc.vector.tensor_copy(out=sel_sb[:], in_=sel_f[:])
            ones_sb = constp.tile([128, 1], BF16)
            nc.vector.memset(ones_sb[:], 1.0)

            for rep_b in range(repeat * BL):
                b = rep_b % BL
                # ---- load x^T for this batch: [128, 6, 1024] ----
                xT_sb = work.tile([128, DT, N], F32R, tag="xT", bufs=1, name="xT_sb")
                nc.sync.dma_start(
                    xT_sb[:],
                    xT_ext[:, b * N:(b + 1) * N]
                    .rearrange("(o p) r -> p o r", p=128)
                    .bitcast(F32R),
                )

                # ---- V projection, row-major bf16: v[k, kb, h, hd] ----
                v_sb = work.tile([128, 8, H, HD], BF16, tag="v", bufs=1, name="v_sb")
                for rb in range(8):
                    for e0, ew in ((0, 512), (512, 256)):
                        vps = mmps.tile([128, 512], F32, tag="mm", name="vps")
                        for di in range(DT):
                            nc.tensor.matmul(
                                vps[:, :ew],
                                xT_sb[:, di, rb * 128:(rb + 1) * 128],
                                wqkvT_sb[:, di, 2 * D + e0:2 * D + e0 + ew],
                                start=(di == 0),
                                stop=(di == DT - 1),
                            )
                        nc.vector.tensor_copy(
                            out=v_sb[:, rb, e0 // HD:(e0 + ew) // HD, :],
                            in_=vps[:, :ew].rearrange("p (h d) -> p h d", d=HD),
                        )

                oT_sb = work.tile([128, NP, N], BF16, tag="oT", bufs=1, name="oT_sb")

                def emit_qk(j):
                    qk_sb = work.tile(
                        [128, 2, N], F32R, tag="qk", bufs=2, name="qk_sb"
                    )
                    for t, e0 in ((0, j * 128), (1, D + j * 128)):
                        for rc in range(2):
                            qps = mmps.tile([128, 512], F32, tag="mm", name="qps")
                            for di in range(DT):
                                nc.tensor.matmul(
                                    qps[:],
                                    wqkvT_sb[:, di, e0:e0 + 128],
                                    xT_sb[:, di, rc * 512:(rc + 1) * 512],
                                    start=(di == 0),
                                    stop=(di == DT - 1),
                                )
                            nc.vector.tensor_copy(
                                out=qk_sb[:, t, rc * 512:(rc + 1) * 512], in_=qps[:]
                            )
                    return qk_sb

                def emit_attention(j, qk_sb):
                    for qc in range(2):  # q chunks of 512
                        qsl = slice(qc * 512, (qc + 1) * 512)
                        # combined pair layout: pT[k, kb, head, q]
                        pT = work.tile(
                            [128, 8, 2, 512], BF16, tag="pT", bufs=2, name="pT"
                        )
                        # S^T + exp, row-tiled head pair (K=64 each); both
                        # heads land in one 2-bank psum tile -> single exp
                        for kb in range(8):
                            ksl = slice(kb * 128, (kb + 1) * 128)
                            stp = stps.tile([128, 1024], F32, tag="stp", name="stp")
                            nc.tensor.matmul(
                                stp[:, 0:512], qk_sb[0:64, 1, ksl], qk_sb[0:64, 0, qsl],
                                start=True, stop=True,
                            )
                            nc.tensor.matmul(
                                stp[:, 512:1024], qk_sb[64:128, 1, ksl], qk_sb[64:128, 0, qsl],
                                start=True, stop=True,
                            )
                            nc.scalar.activation(
                                pT[:, kb, :, :],
                                stp[:].rearrange("p (h q) -> p h q", h=2),
                                EXP, scale=SCALE,
                            )
                        # PV (col-tiled pair) + denominators
                        pv = pvps.tile([128, 512], F32, tag="pv", name="pv")
                        den = denps.tile([33, 512], F32, tag="den", name="den")
                        for kb in range(8):
                            st = (kb == 0)
                            sp = (kb == 7)
                            nc.tensor.matmul(
                                pv[0:64, :], v_sb[:, kb, 2 * j, :], pT[:, kb, 0, :],
                                start=st, stop=sp,
                            )
                            nc.tensor.matmul(
                                pv[64:128, :], v_sb[:, kb, 2 * j + 1, :], pT[:, kb, 1, :],
                                start=st, stop=sp,
                            )
                            nc.tensor.matmul(
                                den[0:1, :], ones_sb[:], pT[:, kb, 0, :],
                                start=st, stop=sp,
                            )
                            nc.tensor.matmul(
                                den[32:33, :], ones_sb[:], pT[:, kb, 1, :],
                                start=st, stop=sp,
                            )
                        # broadcast 1/den over partitions and normalize
                        stage = work.tile([33, 512], F32R, tag="stage", bufs=2, name="stage")
                        nc.vector.tensor_copy(out=stage[0:1, :], in_=den[0:1, :])
                        nc.vector.tensor_copy(out=stage[32:33, :], in_=den[32:33, :])
                        bc = mmps.tile([128, 512], F32, tag="mm", name="bc")
                        nc.tensor.matmul(bc[:], sel_sb[:], stage[:], start=True, stop=True)
                        bcr = work.tile([128, 512], F32, tag="bcr", bufs=2, name="bcr")
                        nc.vector.reciprocal(bcr[:], bc[:])
                        nc.vector.tensor_tensor(
                            oT_sb[:, j, qsl], pv[:], bcr[:], MUL
                        )

                qk_tiles = {0: emit_qk(0)}
                for j in range(NP):  # head pairs (2j, 2j+1)
                    if j + 1 < NP:
                        qk_tiles[j + 1] = emit_qk(j + 1)
                    emit_attention(j, qk_tiles.pop(j))

                # ---- output projection (bf16) + bias ----
                for rb in range(8):
                    out_sb = work.tile([128, D], F32, tag="outsb", bufs=3, name="out_sb")
                    for e0, ew in ((0, 512), (512, 256)):
                        ops = mmps.tile([128, 512], F32, tag="mm", name="ops")
                        for di in range(DT):
                            nc.tensor.matmul(
                                ops[:, :ew],
                                oT_sb[:, di, rb * 128:(rb + 1) * 128],
                                wprojT_sb[:, di, e0:e0 + ew],
                                start=(di == 0),
                                stop=(di == DT - 1),
                            )
                        nc.vector.tensor_tensor(
                            out_sb[:, e0:e0 + ew], ops[:, :ew], bias_sb[:, e0:e0 + ew], ADD
                        )
                    nc.sync.dma_start(
                        out_ext[b * N + rb * 128:b * N + (rb + 1) * 128, :], out_sb[:]
                    )

    nc.compile()
    return nc


_CACHE = {}


def _get_nc():
    if "nc" not in _CACHE:
        _CACHE["nc"] = build_nc()
    return _CACHE["nc"]


def _prep_in_maps(x, w_qkv, w_proj, b_proj):
    x = np.asarray(x, dtype=np.float32)
    w_qkv = np.asarray(w_qkv, dtype=np.float32)
    w_proj = np.asarray(w_proj, dtype=np.float32)
    b_proj = np.asarray(b_proj, dtype=np.float32)

    wqkvT = np.ascontiguousarray(w_qkv.T)                       # [768, 2304]
    wprojT = np.ascontiguousarray(w_proj.T).astype(ml_dtypes.bfloat16)
    biasb = np.ascontiguousarray(np.broadcast_to(b_proj, (128, D)))

    in_maps = []
    for c in range(NCORES):
        xc = x[BL * c:BL * (c + 1)].reshape(ROWS, D)
        in_maps.append({
            "xT": np.ascontiguousarray(xc.T),
            "wqkvT": wqkvT,
            "wprojT": wprojT,
            "biasb": biasb,
        })
    return in_maps


def kernel(x, w_qkv, w_proj, b_proj):
    nc = _get_nc()
    in_maps = _prep_in_maps(x, w_qkv, w_proj, b_proj)
    res = run_bass_kernel_spmd(nc, in_maps, core_ids=list(range(NCORES)))
    out = np.concatenate(
        [res.results[c]["out"].reshape(BL, N, D) for c in range(NCORES)], axis=0
    )
    return out


# revision 6
# speedup vs baseline: 1.0447x; 1.0447x over previous
"""Multi-head attention (B=16, N=1024, D=768, H=12) on 8 TRN2 NeuronCores.

Strategy: data-parallel over batch (2 batches per core, no collectives).
Per-core kernel, all matmuls on TensorE:
  - QKV projection from pre-transposed x (feature-major xT in SBUF),
    fp32r (full-rate fp32-storage matmul mode).
  - Scores computed directly TRANSPOSED (S^T[k, q]) so the exp output
    P^T lands in exactly the layout the PV matmul needs as rhs; the two
    heads of a pair run concurrently on disjoint PE row groups (K=64).
  - exp on ScalarE with the 1/sqrt(hd) scale folded in (no max-subtract:
    scores are O(5) for this input distribution, far from fp32 overflow).
  - Softmax denominators via ones-matmul (M=1 outputs at partition 0/32
    of a shared PSUM bank), broadcast back over partitions with a tiny
    K=33 sel-matmul; the 1/denominator normalization is fused into the
    PV PSUM->SBUF copyback on VectorE.
  - PV col-tiled (two heads per PSUM bank, M=64 each) in bf16 producing
    O^T feature-major, which feeds the output projection (bf16) without
    any transposes.
"""

import sys

sys.path.insert(0, "/opt/trn_rl_repo")

import numpy as np
import ml_dtypes

import concourse.mybir as mybir
import concourse.tile as tile
from concourse import bacc
from concourse.bass_utils import run_bass_kernel_spmd

F32 = mybir.dt.float32
F32R = mybir.dt.float32r
BF16 = mybir.dt.bfloat16

B, N, D = 16, 1024, 768
H = 12
HD = D // H          # 64
SCALE = float(HD) ** -0.5   # 0.125
NCORES = 8
BL = B // NCORES     # batches per core
ROWS = BL * N        # 2048 rows per core
DT = D // 128        # 6 d-tiles
NP = H // 2          # 6 head pairs
EXP = mybir.ActivationFunctionType.Exp
MUL = mybir.AluOpType.mult
ADD = mybir.AluOpType.add


def build_nc(repeat=1):
    nc = bacc.Bacc("TRN2", target_bir_lowering=False, debug=False)

    xT_ext = nc.declare_dram_parameter("xT", [D, ROWS], F32, isOutput=False)
    wqkvT_ext = nc.declare_dram_parameter("wqkvT", [D, 3 * D], F32, isOutput=False)
    wprojT_ext = nc.declare_dram_parameter("wprojT", [D, D], BF16, isOutput=False)
    bias_ext = nc.declare_dram_parameter("biasb", [128, D], F32, isOutput=False)
    out_ext = nc.declare_dram_parameter("out", [ROWS, D], F32, isOutput=True)

    with tile.TileContext(nc) as tc:
        with (
            tc.tile_pool(name="const", bufs=1) as constp,
            tc.tile_pool(name="work", bufs=1) as work,
            tc.tile_pool(name="mmps", bufs=2, space="PSUM") as mmps,
            tc.tile_pool(name="stps", bufs=2, space="PSUM") as stps,
            tc.tile_pool(name="pvps", bufs=1, space="PSUM") as pvps,
            tc.tile_pool(name="denps", bufs=2, space="PSUM") as denps,
        ):
            # ---- constants ----
            wqkvT_sb = constp.tile([128, DT, 3 * D], F32R)
            nc.sync.dma_start(
                wqkvT_sb[:],
                wqkvT_ext.rearrange("(o p) e -> p o e", p=128).bitcast(F32R),
            )
            wprojT_sb = constp.tile([128, DT, D], BF16)
            nc.sync.dma_start(
                wprojT_sb[:], wprojT_ext.rearrange("(o p) e -> p o e", p=128)
            )
            bias_sb = constp.tile([128, D], F32)
            nc.sync.dma_start(bias_sb[:], bias_ext[:])
            sel_f = constp.tile([33, 128], F32)
            nc.vector.memset(sel_f[:], 0.0)
            nc.vector.memset(sel_f[0:1, 0:64], 1.0)
            nc.vector.memset(sel_f[32:33, 64:128], 1.0)
            sel_sb = constp.tile([33, 128], F32R)
            nc.vector.tensor_copy(out=sel_sb[:], in_=sel_f[:])
            ones_sb = constp.tile([128, 1], BF16)
            nc.vector.memset(ones_sb[:], 1.0)

            for rep_b in range(repeat * BL):
                b = rep_b % BL
                # ---- load x^T for this batch: [128, 6, 1024] ----
                xT_sb = work.tile([128, DT, N], F32R, tag="xT", bufs=1, name="xT_sb")
                nc.sync.dma_start(
                    xT_sb[:],
                    xT_ext[:, b * N:(b + 1) * N]
                    .rearrange("(o p) r -> p o r", p=128)
                    .bitcast(F32R),
                )

                # ---- V projection, row-major bf16: v[k, kb, h, hd] ----
                v_sb = work.tile([128, 8, H, HD], BF16, tag="v", bufs=1, name="v_sb")
                for rb in range(8):
                    for e0, ew in ((0, 512), (512, 256)):
                        vps = mmps.tile([128, 512], F32, tag="mm", name="vps")
                        for di in range(DT):
                            nc.tensor.matmul(
                                vps[:, :ew],
                                xT_sb[:, di, rb * 128:(rb + 1) * 128],
                                wqkvT_sb[:, di, 2 * D + e0:2 * D + e0 + ew],
                                start=(di == 0),
                                stop=(di == DT - 1),
                            )
                        nc.vector.tensor_copy(
                            out=v_sb[:, rb, e0 // HD:(e0 + ew) // HD, :],
                            in_=vps[:, :ew].rearrange("p (h d) -> p h d", d=HD),
                        )

                oT_sb = work.tile([128, NP, N], BF16, tag="oT", bufs=1, name="oT_sb")

                def emit_qk(j):
                    qk_sb = work.tile(
                        [128, 2, N], F32R, tag="qk", bufs=2, name="qk_sb"
                    )
                    for t, e0 in ((0, j * 128), (1, D + j * 128)):
                        for rc in range(2):
                            qps = mmps.tile([128, 512], F32, tag="mm", name="qps")
                            for di in range(DT):
                                nc.tensor.matmul(
                                    qps[:],
                                    wqkvT_sb[:, di, e0:e0 + 128],
                                    xT_sb[:, di, rc * 512:(rc + 1) * 512],
                                    start=(di == 0),
                                    stop=(di == DT - 1),
                                )
                            nc.vector.tensor_copy(
                                out=qk_sb[:, t, rc * 512:(rc + 1) * 512], in_=qps[:]
                            )
                    return qk_sb

                def emit_pvden_kb(prev, kb):
                    j, pT, pv, den = prev
                    st = (kb == 0)
                    sp = (kb == 7)
                    nc.tensor.matmul(
                        pv[0:64, :], v_sb[:, kb, 2 * j, :], pT[:, kb, 0, :],
                        start=st, stop=sp,
                    )
                    nc.tensor.matmul(
                        pv[64:128, :], v_sb[:, kb, 2 * j + 1, :], pT[:, kb, 1, :],
                        start=st, stop=sp,
                    )
                    nc.tensor.matmul(
                        den[0:1, :], ones_sb[:], pT[:, kb, 0, :],
                        start=st, stop=sp,
                    )
                    nc.tensor.matmul(
                        den[32:33, :], ones_sb[:], pT[:, kb, 1, :],
                        start=st, stop=sp,
                    )

                def emit_finalize(prev, j, qc):
                    _, pT, pv, den = prev
                    qsl = slice(qc * 512, (qc + 1) * 512)
                    stage = work.tile([33, 512], F32R, tag="stage", bufs=2, name="stage")
                    nc.vector.tensor_copy(out=stage[0:1, :], in_=den[0:1, :])
                    nc.vector.tensor_copy(out=stage[32:33, :], in_=den[32:33, :])
                    bc = denps.tile([128, 512], F32, tag="den", name="bc")
                    nc.tensor.matmul(bc[:], sel_sb[:], stage[:], start=True, stop=True)
                    bcr = work.tile([128, 512], F32, tag="bcr", bufs=2, name="bcr")
                    nc.vector.reciprocal(bcr[:], bc[:])
                    nc.vector.tensor_tensor(
                        oT_sb[:, j, qsl], pv[:], bcr[:], MUL
                    )

                # flat chunk stream (pair, qchunk), software-pipelined:
                # chunk c's S^T/exp interleaved with chunk c-1's PV/den.
                chunks = [(j, qc) for j in range(NP) for qc in range(2)]
                qk_tiles = {0: emit_qk(0)}
                prev = None          # (j, pT, pv, den) awaiting PV/den+finalize
                prev_jqc = None
                for j, qc in chunks:
                    if qc == 1 and j + 1 < NP:
                        qk_tiles[j + 1] = emit_qk(j + 1)
                    qk_sb = qk_tiles[j]
                    qsl = slice(qc * 512, (qc + 1) * 512)
                    pT = work.tile(
                        [128, 8, 2, 512], BF16, tag="pT", bufs=2, name="pT"
                    )
                    for kb in range(8):
                        ksl = slice(kb * 128, (kb + 1) * 128)
                        stp = stps.tile([128, 1024], F32, tag="stp", bufs=1, name="stp")
                        nc.tensor.matmul(
                            stp[:, 0:512], qk_sb[0:64, 1, ksl], qk_sb[0:64, 0, qsl],
                            start=True, stop=True,
                        )
                        nc.tensor.matmul(
                            stp[:, 512:1024], qk_sb[64:128, 1, ksl], qk_sb[64:128, 0, qsl],
                            start=True, stop=True,
                        )
                        if prev is not None:
                            emit_pvden_kb(prev, kb)
                        nc.scalar.activation(
                            pT[:, kb, :, :],
                            stp[:].rearrange("p (h q) -> p h q", h=2),
                            EXP, scale=SCALE,
                        )
                    if prev is not None:
                        emit_finalize(prev, *prev_jqc)
                    pv = pvps.tile([128, 512], F32, tag="pv", name="pv")
                    den = denps.tile([33, 512], F32, tag="den", name="den")
                    prev = (j, pT, pv, den)
                    prev_jqc = (j, qc)
                    if (j, qc) == chunks[-1]:
                        for kb in range(8):
                            emit_pvden_kb(prev, kb)
                        emit_finalize(prev, *prev_jqc)
                        prev = None

                # ---- output projection (bf16) + bias ----
                for rb in range(8):
                    out_sb = work.tile([128, D], F32, tag="outsb", bufs=3, name="out_sb")
                    for e0, ew in ((0, 512), (512, 256)):
                        ops = mmps.tile([128, 512], F32, tag="mm", name="ops")
                        for di in range(DT):
                            nc.tensor.matmul(
                                ops[:, :ew],
                                oT_sb[:, di, rb * 128:(rb + 1) * 128],
                                wprojT_sb[:, di, e0:e0 + ew],
                                start=(di == 0),
                                stop=(di == DT - 1),
                            )
                        nc.vector.tensor_tensor(
                            out_sb[:, e0:e0 + ew], ops[:, :ew], bias_sb[:, e0:e0 + ew], ADD
                        )
                    nc.sync.dma_start(
                        out_ext[b * N + rb * 128:b * N + (rb + 1) * 128, :], out_sb[:]
                    )

    nc.compile()
    return nc


_CACHE = {}


def _get_nc():
    if "nc" not in _CACHE:
        _CACHE["nc"] = build_nc()
    return _CACHE["nc"]


def _prep_in_maps(x, w_qkv, w_proj, b_proj):
    x = np.asarray(x, dtype=np.float32)
    w_qkv = np.asarray(w_qkv, dtype=np.float32)
    w_proj = np.asarray(w_proj, dtype=np.float32)
    b_proj = np.asarray(b_proj, dtype=np.float32)

    wqkvT = np.ascontiguousarray(w_qkv.T)                       # [768, 2304]
    wprojT = np.ascontiguousarray(w_proj.T).astype(ml_dtypes.bfloat16)
    biasb = np.ascontiguousarray(np.broadcast_to(b_proj, (128, D)))

    in_maps = []
    for c in range(NCORES):
        xc = x[BL * c:BL * (c + 1)].reshape(ROWS, D)
        in_maps.append({
            "xT": np.ascontiguousarray(xc.T),
            "wqkvT": wqkvT,
            "wprojT": wprojT,
            "biasb": biasb,
        })
    return in_maps


def kernel(x, w_qkv, w_proj, b_proj):
    nc = _get_nc()
    in_maps = _prep_in_maps(x, w_qkv, w_proj, b_proj)
    res = run_bass_kernel_spmd(nc, in_maps, core_ids=list(range(NCORES)))
    out = np.concatenate(
        [res.results[c]["out"].reshape(BL, N, D) for c in range(NCORES)], axis=0
    )
    return out


# revision 7
# speedup vs baseline: 1.1811x; 1.1306x over previous
"""Multi-head attention (B=16, N=1024, D=768, H=12) on 8 TRN2 NeuronCores.

Strategy: data-parallel over batch (2 batches per core, no collectives).
Per-core kernel, all matmuls on TensorE:
  - QKV projection from pre-transposed x (feature-major xT in SBUF),
    fp32r (full-rate fp32-storage matmul mode).
  - Scores computed directly TRANSPOSED (S^T[k, q]) so the exp output
    P^T lands in exactly the layout the PV matmul needs as rhs; the two
    heads of a pair run concurrently on disjoint PE row groups (K=64).
  - exp on ScalarE with the 1/sqrt(hd) scale folded in (no max-subtract:
    scores are O(5) for this input distribution, far from fp32 overflow).
  - Softmax denominators via ones-matmul (M=1 outputs at partition 0/32
    of a shared PSUM bank), broadcast back over partitions with a tiny
    K=33 sel-matmul; the 1/denominator normalization is fused into the
    PV PSUM->SBUF copyback on VectorE.
  - PV col-tiled (two heads per PSUM bank, M=64 each) in bf16 producing
    O^T feature-major, which feeds the output projection (bf16) without
    any transposes.
"""

import sys

sys.path.insert(0, "/opt/trn_rl_repo")

import numpy as np
import ml_dtypes

import concourse.mybir as mybir
import concourse.tile as tile
from concourse import bacc
from concourse.bass_utils import run_bass_kernel_spmd

F32 = mybir.dt.float32
F32R = mybir.dt.float32r
BF16 = mybir.dt.bfloat16

B, N, D = 16, 1024, 768
H = 12
HD = D // H          # 64
SCALE = float(HD) ** -0.5   # 0.125
NCORES = 8
BL = B // NCORES     # batches per core
ROWS = BL * N        # 2048 rows per core
DT = D // 128        # 6 d-tiles
NP = H // 2          # 6 head pairs
EXP = mybir.ActivationFunctionType.Exp
MUL = mybir.AluOpType.mult
ADD = mybir.AluOpType.add


def build_nc(repeat=1, qk_bf16=False):
    nc = bacc.Bacc("TRN2", target_bir_lowering=False, debug=False)

    QKDT = BF16 if qk_bf16 else F32R
    xT_ext = nc.declare_dram_parameter("xT", [D, ROWS], BF16 if qk_bf16 else F32, isOutput=False)
    wqkvT_ext = nc.declare_dram_parameter("wqkvT", [D, 3 * D], BF16 if qk_bf16 else F32, isOutput=False)
    wprojT_ext = nc.declare_dram_parameter("wprojT", [D, D], BF16, isOutput=False)
    bias_ext = nc.declare_dram_parameter("biasb", [128, D], F32, isOutput=False)
    out_ext = nc.declare_dram_parameter("out", [ROWS, D], F32, isOutput=True)

    with tile.TileContext(nc) as tc:
        with (
            tc.tile_pool(name="const", bufs=1) as constp,
            tc.tile_pool(name="work", bufs=1) as work,
            tc.tile_pool(name="mmps", bufs=2, space="PSUM") as mmps,
            tc.tile_pool(name="stps", bufs=2, space="PSUM") as stps,
            tc.tile_pool(name="pvps", bufs=1, space="PSUM") as pvps,
            tc.tile_pool(name="denps", bufs=2, space="PSUM") as denps,
        ):
            # ---- constants ----
            wqkvT_sb = constp.tile([128, DT, 3 * D], QKDT)
            wq_src = wqkvT_ext.rearrange("(o p) e -> p o e", p=128)
            nc.sync.dma_start(
                wqkvT_sb[:], wq_src if qk_bf16 else wq_src.bitcast(F32R)
            )
            wprojT_sb = constp.tile([128, DT, D], BF16)
            nc.sync.dma_start(
                wprojT_sb[:], wprojT_ext.rearrange("(o p) e -> p o e", p=128)
            )
            bias_sb = constp.tile([128, D], F32)
            nc.sync.dma_start(bias_sb[:], bias_ext[:])
            sel_f = constp.tile([33, 128], F32)
            nc.vector.memset(sel_f[:], 0.0)
            nc.vector.memset(sel_f[0:1, 0:64], 1.0)
            nc.vector.memset(sel_f[32:33, 64:128], 1.0)
            sel_sb = constp.tile([33, 128], QKDT)
            nc.vector.tensor_copy(out=sel_sb[:], in_=sel_f[:])
            ones_sb = constp.tile([128, 1], BF16)
            nc.vector.memset(ones_sb[:], 1.0)

            for rep_b in range(repeat * BL):
                b = rep_b % BL
                # ---- load x^T for this batch: [128, 6, 1024] ----
                xT_sb = work.tile([128, DT, N], QKDT, tag="xT", bufs=1, name="xT_sb")
                xT_src = xT_ext[:, b * N:(b + 1) * N].rearrange("(o p) r -> p o r", p=128)
                nc.sync.dma_start(
                    xT_sb[:], xT_src if qk_bf16 else xT_src.bitcast(F32R)
                )

                # ---- V projection, row-major bf16: v[k, kb, h, hd] ----
                v_sb = work.tile([128, 8, H, HD], BF16, tag="v", bufs=1, name="v_sb")
                for rb in range(8):
                    for e0, ew in ((0, 512), (512, 256)):
                        vps = mmps.tile([128, 512], F32, tag="mm", name="vps")
                        for di in range(DT):
                            nc.tensor.matmul(
                                vps[:, :ew],
                                xT_sb[:, di, rb * 128:(rb + 1) * 128],
                                wqkvT_sb[:, di, 2 * D + e0:2 * D + e0 + ew],
                                start=(di == 0),
                                stop=(di == DT - 1),
                            )
                        nc.vector.tensor_copy(
                            out=v_sb[:, rb, e0 // HD:(e0 + ew) // HD, :],
                            in_=vps[:, :ew].rearrange("p (h d) -> p h d", d=HD),
                        )

                oT_sb = work.tile([128, NP, N], BF16, tag="oT", bufs=1, name="oT_sb")

                def emit_qk(j):
                    qk_sb = work.tile(
                        [128, 2, N], QKDT, tag="qk", bufs=2, name="qk_sb"
                    )
                    for t, e0 in ((0, j * 128), (1, D + j * 128)):
                        for rc in range(2):
                            qps = mmps.tile([128, 512], F32, tag="mm", name="qps")
                            for di in range(DT):
                                nc.tensor.matmul(
                                    qps[:],
                                    wqkvT_sb[:, di, e0:e0 + 128],
                                    xT_sb[:, di, rc * 512:(rc + 1) * 512],
                                    start=(di == 0),
                                    stop=(di == DT - 1),
                                )
                            nc.vector.tensor_copy(
                                out=qk_sb[:, t, rc * 512:(rc + 1) * 512], in_=qps[:]
                            )
                    return qk_sb

                def emit_pvden_kb(prev, kb):
                    j, pT, pv, den = prev
                    st = (kb == 0)
                    sp = (kb == 7)
                    nc.tensor.matmul(
                        pv[0:64, :], v_sb[:, kb, 2 * j, :], pT[:, kb, 0, :],
                        start=st, stop=sp,
                    )
                    nc.tensor.matmul(
                        pv[64:128, :], v_sb[:, kb, 2 * j + 1, :], pT[:, kb, 1, :],
                        start=st, stop=sp,
                    )
                    nc.tensor.matmul(
                        den[0:1, :], ones_sb[:], pT[:, kb, 0, :],
                        start=st, stop=sp,
                    )
                    nc.tensor.matmul(
                        den[32:33, :], ones_sb[:], pT[:, kb, 1, :],
                        start=st, stop=sp,
                    )

                def emit_finalize(prev, j, qc):
                    _, pT, pv, den = prev
                    qsl = slice(qc * 512, (qc + 1) * 512)
                    stage = work.tile([33, 512], QKDT, tag="stage", bufs=2, name="stage")
                    nc.vector.tensor_copy(out=stage[0:1, :], in_=den[0:1, :])
                    nc.vector.tensor_copy(out=stage[32:33, :], in_=den[32:33, :])
                    bc = denps.tile([128, 512], F32, tag="den", name="bc")
                    nc.tensor.matmul(bc[:], sel_sb[:], stage[:], start=True, stop=True)
                    bcr = work.tile([128, 512], F32, tag="bcr", bufs=2, name="bcr")
                    nc.vector.reciprocal(bcr[:], bc[:])
                    nc.vector.tensor_tensor(
                        oT_sb[:, j, qsl], pv[:], bcr[:], MUL
                    )

                # flat chunk stream (pair, qchunk), software-pipelined:
                # chunk c's S^T/exp interleaved with chunk c-1's PV/den.
                chunks = [(j, qc) for j in range(NP) for qc in range(2)]
                qk_tiles = {0: emit_qk(0)}
                prev = None          # (j, pT, pv, den) awaiting PV/den+finalize
                prev_jqc = None
                for j, qc in chunks:
                    if qc == 1 and j + 1 < NP:
                        qk_tiles[j + 1] = emit_qk(j + 1)
                    qk_sb = qk_tiles[j]
                    qsl = slice(qc * 512, (qc + 1) * 512)
                    pT = work.tile(
                        [128, 8, 2, 512], BF16, tag="pT", bufs=2, name="pT"
                    )
                    for kb in range(8):
                        ksl = slice(kb * 128, (kb + 1) * 128)
                        stp = stps.tile([128, 1024], F32, tag="stp", bufs=1, name="stp")
                        nc.tensor.matmul(
                            stp[:, 0:512], qk_sb[0:64, 1, ksl], qk_sb[0:64, 0, qsl],
                            start=True, stop=True,
                        )
                        nc.tensor.matmul(
                            stp[:, 512:1024], qk_sb[64:128, 1, ksl], qk_sb[64:128, 0, qsl],
                            start=True, stop=True,
                        )
                        if prev is not None:
                            emit_pvden_kb(prev, kb)
                        nc.scalar.activation(
                            pT[:, kb, :, :],
                            stp[:].rearrange("p (h q) -> p h q", h=2),
                            EXP, scale=SCALE,
                        )
                    if prev is not None:
                        emit_finalize(prev, *prev_jqc)
                    pv = pvps.tile([128, 512], F32, tag="pv", name="pv")
                    den = denps.tile([33, 512], F32, tag="den", name="den")
                    prev = (j, pT, pv, den)
                    prev_jqc = (j, qc)
                    if (j, qc) == chunks[-1]:
                        for kb in range(8):
                            emit_pvden_kb(prev, kb)
                        emit_finalize(prev, *prev_jqc)
                        prev = None

                # ---- output projection (bf16) + bias ----
                for rb in range(8):
                    out_sb = work.tile([128, D], F32, tag="outsb", bufs=3, name="out_sb")
                    for e0, ew in ((0, 512), (512, 256)):
                        ops = mmps.tile([128, 512], F32, tag="mm", name="ops")
                        for di in range(DT):
                            nc.tensor.matmul(
                                ops[:, :ew],
                                oT_sb[:, di, rb * 128:(rb + 1) * 128],
                                wprojT_sb[:, di, e0:e0 + ew],
                                start=(di == 0),
                                stop=(di == DT - 1),
                            )
                        nc.vector.tensor_tensor(
                            out_sb[:, e0:e0 + ew], ops[:, :ew], bias_sb[:, e0:e0 + ew], ADD
                        )
                    nc.sync.dma_start(
                        out_ext[b * N + rb * 128:b * N + (rb + 1) * 128, :], out_sb[:]
                    )

    nc.compile()
    return nc


_CACHE = {}


def _get_nc():
    if "nc" not in _CACHE:
        _CACHE["nc"] = build_nc()
    return _CACHE["nc"]


def _prep_in_maps(x, w_qkv, w_proj, b_proj, qk_bf16=False):
    x = np.asarray(x, dtype=np.float32)
    w_qkv = np.asarray(w_qkv, dtype=np.float32)
    w_proj = np.asarray(w_proj, dtype=np.float32)
    b_proj = np.asarray(b_proj, dtype=np.float32)

    wqkvT = np.ascontiguousarray(w_qkv.T)                       # [768, 2304]
    if qk_bf16:
        wqkvT = wqkvT.astype(ml_dtypes.bfloat16)
    wprojT = np.ascontiguousarray(w_proj.T).astype(ml_dtypes.bfloat16)
    biasb = np.ascontiguousarray(np.broadcast_to(b_proj, (128, D)))

    in_maps = []
    for c in range(NCORES):
        xc = x[BL * c:BL * (c + 1)].reshape(ROWS, D)
        xTc = np.ascontiguousarray(xc.T)
        if qk_bf16:
            xTc = xTc.astype(ml_dtypes.bfloat16)
        in_maps.append({
            "xT": xTc,
            "wqkvT": wqkvT,
            "wprojT": wprojT,
            "biasb": biasb,
        })
    return in_maps


def kernel(x, w_qkv, w_proj, b_proj):
    nc = _get_nc()
    in_maps = _prep_in_maps(x, w_qkv, w_proj, b_proj)
    res = run_bass_kernel_spmd(nc, in_maps, core_ids=list(range(NCORES)))
    out = np.concatenate(
        [res.results[c]["out"].reshape(BL, N, D) for c in range(NCORES)], axis=0
    )
    return out
